# revision 3
# baseline (speedup 1.0000x reference)
"""Causal MHA with RoPE on 8 TRN2 NeuronCores — fp8/fp16 redesign.

Sharding: core c -> batch c//4, heads [4*(c%4), 4*(c%4)+4). Host sums the 4
partial output projections per batch.

Key structure vs the bf16 baseline:
- QKV projection in fp8e4m3 DoubleRow with unscaled residual splitting
  (x=xh+xl, W=wh+wl, three products accumulated in one PSUM group): 4x fewer
  PE rows than bf16 at ~bf16 accuracy.
- Scores in bf16 (q/k from RoPE), per-head 64-partition stationary tiles.
- Softmax probs in fp16. exp is split across engines: Activation runs true
  exp; DVE/Pool run a Schraudolph exp (tensor_scalar mult+add writing int16
  that bitcasts to fp16).
- PV transposed: stationary probs [keys, q128], moving V[keys, 65] with a
  WS-valued ones column producing the denominator per q-partition; normalize
  is a per-partition reciprocal+broadcast mul; PE transposes route the
  attention output into feature-major o2 for the bf16 output projection.
"""

import math
import sys

sys.path.insert(0, "/opt/trn_rl_repo")

import numpy as np
import ml_dtypes

import concourse.bass as bass
import concourse.bacc as bacc
import concourse.mybir as mybir
import concourse.tile as tile
from concourse.bass_utils import run_bass_kernel_spmd

B, S, D = 2, 2048, 1024
H, DK = 16, 64
THETA = 10000.0
HPC = 4
N_CORES = 8
P = 128
QTILE = 512
NQ = S // QTILE      # 4 phases
NKT = S // P         # 16 key tiles
NJ = S // 256        # 8 q256 tiles
BF = ml_dtypes.bfloat16
F8NP = ml_dtypes.float8_e4m3
F16NP = np.float16

WS = 32.0
ESCALE = 0.125 / (WS * WS)
A16 = ESCALE * 1024.0 / math.log(2.0)
B16 = 15360.5

_PAIRSWAP = [i + 1 if i % 2 == 0 else i - 1 for i in range(32)]

_CACHE = {}


def _build_nc(reps=1, loop=False, probe=(), opts=None):
    probe = set(probe)
    opts = dict(opts or {})
    prab_bufs = opts.get("prab_bufs", 36)
    pvlag = opts.get("pvlag", 5)
    rope_bufs = opts.get("rope_bufs", 6)
    ysb_bufs = opts.get("ysb_bufs", 6)
    exp_cycle = opts.get("exp_cycle", "aad")  # a=Act d=DVE per (j,kt); Pool
    exp_sched = opts.get(                     # cannot touch PSUM on TRN2
        "exp_sched", {0: "aaad", 1: "aaad", 2: "aadaad", 3: "aadadad"})
    ysb_cycle = opts.get("ysb_cycle", "ad")
    xcopy_eng = opts.get("xcopy_eng", "a")
    vcopy_eng = opts.get("vcopy_eng", "a")
    mask_eng = opts.get("mask_eng", "d")
    ropeadd_eng = opts.get("ropeadd_eng", "p")
    ropemul_eng = opts.get("ropemul_eng", "d")
    o2_eng = opts.get("o2_eng", "d")
    sc_bufs = opts.get("sc_bufs", 2)
    mid_bufs = opts.get("mid_bufs", 2)
    po_bufs = opts.get("po_bufs", 2)
    f32 = mybir.dt.float32
    bf16 = mybir.dt.bfloat16
    fp16 = mybir.dt.float16
    fp8 = mybir.dt.float8e4
    i16 = mybir.dt.int16
    Exp = mybir.ActivationFunctionType.Exp
    DR = mybir.MatmulPerfMode.DoubleRow

    nc = bacc.Bacc()
    xh_d = nc.dram_tensor("xh", [P, 4, 2, S], fp8, kind="ExternalInput")
    xl_d = nc.dram_tensor("xl", [P, 4, 2, S], fp8, kind="ExternalInput")
    wqkh_d = nc.dram_tensor("wqkh", [P, 4, 2, 512], fp8, kind="ExternalInput")
    wqkl_d = nc.dram_tensor("wqkl", [P, 4, 2, 512], fp8, kind="ExternalInput")
    wvh_d = nc.dram_tensor("wvh", [P, 4, 2, 256], fp8, kind="ExternalInput")
    wvl_d = nc.dram_tensor("wvl", [P, 4, 2, 256], fp8, kind="ExternalInput")
    wo_d = nc.dram_tensor("wo", [P, 2, D], bf16, kind="ExternalInput")
    cos_d = nc.dram_tensor("cosv", [P, S], bf16, kind="ExternalInput")
    sin_d = nc.dram_tensor("sinv", [P, S], bf16, kind="ExternalInput")
    mask_d = nc.dram_tensor("masks", [P, P], bf16, kind="ExternalInput")
    ident_d = nc.dram_tensor("ident", [P, P], fp16, kind="ExternalInput")
    out_d = nc.dram_tensor("out_t", [D, S], bf16, kind="ExternalOutput")

    with tile.TileContext(nc) as tc:
        with (
            tc.tile_pool(name="const", bufs=1) as cp,
            tc.tile_pool(name="rope", bufs=rope_bufs) as ropep,
            tc.tile_pool(name="probs", bufs=prab_bufs) as probsp,
            tc.tile_pool(name="small", bufs=4) as smallp,
            tc.tile_pool(name="ysb", bufs=ysb_bufs) as ysbp,
            tc.tile_pool(name="ps_sc", bufs=sc_bufs, space="PSUM") as ps_sc,
            tc.tile_pool(name="ps_mid", bufs=mid_bufs, space="PSUM") as ps_mid,
            tc.tile_pool(name="ps_po", bufs=po_bufs, space="PSUM") as ps_po,
        ):
            xh = cp.tile([P, 4, 2, S], fp8, tag="xh")
            xl = cp.tile([P, 4, 2, S], fp8, tag="xl")
            wqkh = cp.tile([P, 4, 2, 512], fp8, tag="wqkh")
            wqkl = cp.tile([P, 4, 2, 512], fp8, tag="wqkl")
            wvh = cp.tile([P, 4, 2, 256], fp8, tag="wvh")
            wvl = cp.tile([P, 4, 2, 256], fp8, tag="wvl")
            wo = cp.tile([P, 2, D], bf16, tag="wo")
            cosv = cp.tile([P, S], bf16, tag="cos")
            sinv = cp.tile([P, S], bf16, tag="sin")
            maskt = cp.tile([P, P], bf16, tag="mask")
            ident = cp.tile([P, P], fp16, tag="ident")
            qsb = cp.tile([P, 2, S], bf16, tag="qsb")
            ksb = cp.tile([P, 2, S], bf16, tag="ksb")
            v2 = cp.tile([P, NKT, HPC, 65], fp16, tag="v2")
            o2 = cp.tile([P, 2, S], fp16, tag="o2")

            # Act table warmup during the input-DMA wait
            warm = smallp.tile([P, 2], f32, tag="warm", name="warm")
            nc.vector.memset(warm[:], 0.0)
            nc.scalar.activation(warm[:, 0:1], warm[:, 1:2],
                                 Exp, scale=1.0)

            # input DMAs: few big transfers, first QKV chain's first
            sl0 = slice(0, QTILE)
            for kk in range(2):
                ksl = slice(kk * 2, kk * 2 + 2)
                nc.sync.dma_start(wqkh[:, ksl, :, :], wqkh_d[:, ksl, :, :])
                nc.sync.dma_start(xh[:, ksl, :, sl0], xh_d[:, ksl, :, sl0])
            for kk in range(2):
                ksl = slice(kk * 2, kk * 2 + 2)
                nc.sync.dma_start(xl[:, ksl, :, sl0], xl_d[:, ksl, :, sl0])
            nc.sync.dma_start(wqkl[:], wqkl_d[:])
            nc.sync.dma_start(cosv[:, sl0], cos_d[:, sl0])
            nc.sync.dma_start(sinv[:, sl0], sin_d[:, sl0])
            nc.sync.dma_start(wvh[:], wvh_d[:])
            nc.sync.dma_start(wvl[:], wvl_d[:])
            for t in range(1, NQ):
                sl = slice(t * QTILE, (t + 1) * QTILE)
                nc.sync.dma_start(xh[:, :, :, sl], xh_d[:, :, :, sl])
                nc.sync.dma_start(xl[:, :, :, sl], xl_d[:, :, :, sl])
                nc.sync.dma_start(cosv[:, sl], cos_d[:, sl])
                nc.sync.dma_start(sinv[:, sl], sin_d[:, sl])
                if t == 1:
                    nc.sync.dma_start(maskt[:], mask_d[:])
                    nc.sync.dma_start(ident[:], ident_d[:])
            nc.sync.dma_start(wo[:], wo_d[:])
            nc.gpsimd.memset(v2[:], WS)

            exp_n = [0]
            ysb_n = [0]

            def qkv_dr(out_ap, stat_tile_pair, mov_tile_pair, stat_sl, mov_sl):
                """12 DoubleRow matmuls: 3 residual products x 4 ksteps."""
                prods = [
                    (stat_tile_pair[0], mov_tile_pair[0]),
                    (stat_tile_pair[0], mov_tile_pair[1]),
                    (stat_tile_pair[1], mov_tile_pair[0]),
                ]
                n = len(prods) * 4
                i = 0
                for st, mv in prods:
                    for ks in range(4):
                        nc.tensor.matmul(
                            out_ap,
                            st[(slice(None), ks) + stat_sl],
                            mv[(slice(None), ks) + mov_sl],
                            start=(i == 0),
                            stop=(i == n - 1),
                            perf_mode=DR,
                        )
                        i += 1

            def qkv_unit(t, mt):
                """one QKV mt-tile (q or k head-pair) + its rope chain"""
                nsl = slice(t * QTILE, (t + 1) * QTILE)
                pqk = ps_mid.tile([P, QTILE], f32, tag="m", name="pqk")
                if "noqkv" in probe:
                    nc.vector.memset(pqk[:], 0.01)
                else:
                    qkv_dr(
                        pqk[:],
                        (wqkh, wqkl),
                        (xh, xl),
                        (slice(None), slice(mt * 128, (mt + 1) * 128)),
                        (slice(None), nsl),
                    )
                # rope: copy on Act, shuffle+muls+add on DVE (bf16 2x)
                cpy = ropep.tile([P, QTILE], bf16, tag="cpy", name="cpy")
                if xcopy_eng == "a":
                    nc.scalar.copy(cpy[:], pqk[:])
                else:
                    nc.vector.tensor_copy(cpy[:], pqk[:])
                sw = ropep.tile([P, QTILE], bf16, tag="sw", name="sw")
                nc.vector.stream_shuffle(sw[:], cpy[:], _PAIRSWAP)
                t0 = ropep.tile([P, QTILE], bf16, tag="t0", name="t0")
                nc.vector.tensor_mul(t0[:], cpy[:], cosv[:, nsl])
                u0 = ropep.tile([P, QTILE], bf16, tag="u0", name="u0")
                rm = nc.gpsimd if ropemul_eng == "p" else nc.vector
                rm.tensor_mul(u0[:], sw[:], sinv[:, nsl])
                dst = qsb if mt < 2 else ksb
                radd = nc.gpsimd if ropeadd_eng == "p" else nc.vector
                radd.tensor_add(dst[:, mt % 2, nsl], t0[:], u0[:])

            def v_unit(t, tt):
                kt = t * 4 + tt
                pv = ps_mid.tile([P, 2, HPC, DK], f32, tag="m", name="pv")
                if "noqkv" in probe:
                    nc.vector.memset(pv[:, 0, :, :], 0.01)
                else:
                    qkv_dr(
                        pv[:, 0, :, :],
                        (xh, xl),
                        (wvh, wvl),
                        (slice(None), slice(kt * P, (kt + 1) * P)),
                        (slice(None), slice(None)),
                    )
                if vcopy_eng == "a":
                    nc.scalar.copy(v2[:, kt, :, 0:DK], pv[:, 0, :, :])
                else:
                    nc.vector.tensor_copy(v2[:, kt, :, 0:DK], pv[:, 0, :, :])

            prabs = {}

            def sc_unit(qt, kt, hp):
                """scores+exp(+mask) for one (kt, head-pair)"""
                if True:
                    ktsl = slice(kt * P, (kt + 1) * P)
                    r = kt - 4 * qt  # 0..3 diag band; negative for full kts
                    rq = max(r, 0) * 128
                    qsl = slice(qt * QTILE + rq, (qt + 1) * QTILE)
                    if True:
                        pst = ps_sc.tile([P, 2, QTILE], f32, tag="s", name="pst")
                        if "noscores" in probe:
                            nc.vector.memset(pst[:, :, rq:rq + 1], 0.0)
                        else:
                            for half in range(2):
                                psl = slice(half * 64, half * 64 + 64)
                                nc.tensor.matmul(
                                    pst[:, half, rq:],
                                    ksb[psl, hp, ktsl],
                                    qsb[psl, hp, qsl],
                                    start=True,
                                    stop=True,
                                    tile_position=(half * 64, 0),
                                )
                        prab = probsp.tile([P, 2, QTILE], fp16, tag="pr", name="pr")
                        cyc = exp_sched.get(qt, exp_cycle)
                        eng = cyc[exp_n[0] % len(cyc)]
                        exp_n[0] += 1
                        if "noexp" in probe:
                            nc.vector.memset(prab[:, :, rq:rq + 1], 0.001)
                        elif eng == "a":
                            nc.scalar.activation(
                                prab[:, :, rq:], pst[:, :, rq:], Exp, scale=ESCALE
                            )
                        else:
                            e = nc.vector if eng == "d" else nc.gpsimd
                            e.tensor_scalar(
                                prab[:, :, rq:].bitcast(i16),
                                pst[:, :, rq:],
                                A16,
                                B16,
                                op0=mybir.AluOpType.mult,
                                op1=mybir.AluOpType.add,
                            )
                        if r >= 0 and "nomask" not in probe:
                            meng = nc.gpsimd if mask_eng == "p" else nc.vector
                            meng.tensor_mul(
                                prab[:, :, rq:rq + P],
                                prab[:, :, rq:rq + P],
                                maskt[:, None, :].to_broadcast([P, 2, P]),
                            )
                        prabs[(kt, hp)] = prab

            def pv_unit(qt, qi):
                # PV + normalize for one q128; sequential per-(q128, head)
                # chains so each po bank has one open PSUM group at a time
                if True:
                    c = qi % 4
                    po = ps_po.tile([P, HPC, 128], f32, tag="po", name="po")
                    if "nopv" in probe:
                        nc.vector.memset(po[:], 1.0)
                    else:
                        for h in range(HPC):
                            for kt in range(qi + 1):
                                nc.tensor.matmul(
                                    po[:, h, 0:65],
                                    prabs[(kt, h // 2)][
                                        :, h % 2, c * 128:(c + 1) * 128
                                    ],
                                    v2[:, kt, h, :],
                                    start=(kt == 0),
                                    stop=(kt == qi),
                                )
                    if "nonorm" in probe:
                        nc.vector.memset(o2[:, :, qi * P:(qi + 1) * P], 0.01)
                        return
                    rcp = smallp.tile([P, HPC], f32, tag="rc", name="rcp")
                    nc.vector.reciprocal(rcp[:], po[:, :, 64:65])
                    onorm = smallp.tile([P, HPC, DK], fp16, tag="on", name="onorm")
                    nc.vector.tensor_mul(
                        onorm[:],
                        po[:, :, 0:DK],
                        rcp[:, :, None].to_broadcast([P, HPC, DK]),
                    )
                    if "notr" in probe:
                        nc.vector.memset(o2[:, :, qi * P:(qi + 1) * P], 0.01)
                        return
                    tr = ps_mid.tile([P, 2, 256], f32, tag="m", name="tr")
                    trh = tr[:].bitcast(fp16)  # [P, 2, 512] fp16 view
                    for kj in range(2):
                        nc.tensor.transpose(
                            trh[:, kj, 0:128],
                            onorm[:, 2 * kj:2 * kj + 2, :],
                            ident[:],
                        )
                    if o2_eng == "a":
                        nc.scalar.copy(
                            o2[:, :, qi * P:(qi + 1) * P], trh[:, :, 0:128])
                    else:
                        nc.vector.tensor_copy(
                            o2[:, :, qi * P:(qi + 1) * P], trh[:, :, 0:128])

            def oproj_unit(t, ot, qsl=None):
                qsl = qsl or slice(t * QTILE, (t + 1) * QTILE)
                w = qsl.stop - qsl.start
                py = ps_mid.tile([P, QTILE], f32, tag="m", name="py")
                for kj in range(2):
                    nc.tensor.matmul(
                        py[:, 0:w],
                        wo[:, kj, ot * 128:(ot + 1) * 128],
                        o2[:, kj, qsl],
                        start=(kj == 0),
                        stop=(kj == 1),
                    )
                if "noy" in probe:
                    return
                ysb = ysbp.tile([P, QTILE], bf16, tag="y", name="ysb")
                eng = ysb_cycle[ysb_n[0] % len(ysb_cycle)]
                ysb_n[0] += 1
                if eng == "a":
                    nc.scalar.copy(ysb[:, 0:w], py[:, 0:w])
                elif eng == "d":
                    nc.vector.tensor_copy(ysb[:, 0:w], py[:, 0:w])
                else:
                    nc.gpsimd.tensor_copy(ysb[:, 0:w], py[:, 0:w])
                nc.sync.dma_start(
                    out_d[ot * 128:(ot + 1) * 128, qsl], ysb[:, 0:w])

            def body():
                if "noattn" in probe:
                    nc.vector.memset(o2[:], 0.01)
                # phase 0 QKV up front
                for mt in range(4):
                    qkv_unit(0, mt)
                for tt in range(4):
                    v_unit(0, tt)
                for t in range(NQ):
                    # fill units: next phase's QKV/V, prev phase's oproj
                    fill = []
                    if t + 1 < NQ:
                        fill += [(qkv_unit, (t + 1, mt)) for mt in range(4)]
                        fill += [(v_unit, (t + 1, tt)) for tt in range(4)]
                    if t > 0:
                        fill += [(oproj_unit, (t - 1, ot)) for ot in range(8)]
                    # interleave: scores stream + PV chains + fill units
                    nkt = 4 * (t + 1)
                    fi = 0
                    emitted_pv = 0
                    if "noattn" in probe:
                        seq = []
                    else:
                        seq = [(sc_unit, (t, kt, hp))
                               for kt in range(nkt) for hp in range(2)]
                    for n, (fn, args) in enumerate(seq):
                        fn(*args)
                        kt = args[1]
                        # a fill unit after every sc pair
                        if n % 2 == 1 and fi < len(fill):
                            fn2, a2 = fill[fi]
                            fn2(*a2)
                            fi += 1
                        # PV chain for qi once sc(kt=qi+pvlag) has been emitted
                        while (emitted_pv < 4
                               and 4 * t + emitted_pv + pvlag <= kt):
                            pv_unit(t, 4 * t + emitted_pv)
                            emitted_pv += 1
                    while emitted_pv < 4 and "noattn" not in probe:
                        pv_unit(t, 4 * t + emitted_pv)
                        emitted_pv += 1
                    while fi < len(fill):
                        fn2, a2 = fill[fi]
                        fn2(*a2)
                        fi += 1
                    prabs.clear()
                # last oproj
                for ot in range(8):
                    oproj_unit(NQ - 1, ot)

            if loop:
                with tc.For_i(0, reps, 1):
                    body()
            else:
                for _rep in range(reps):
                    body()
    nc.compile()
    return nc


def _f8(a):
    return np.asarray(a, dtype=F8NP)


def _dr_major(Wmat):
    """[M, 1024] -> [128, 4, 2, M]: (p, ks, i, m) = W[m, ks*256+i*128+p]."""
    M = Wmat.shape[0]
    return np.ascontiguousarray(
        Wmat.T.reshape(4, 2, P, M).transpose(2, 0, 1, 3)
    )


def _prep_in_maps(x, W_qkv, W_o, token_positions):
    x = np.asarray(x, dtype=np.float32)
    W_qkv = np.asarray(W_qkv, dtype=np.float32)
    W_o = np.asarray(W_o, dtype=np.float32)
    pos = np.asarray(token_positions)

    inv_freq = 1.0 / (
        np.float32(THETA) ** (np.arange(0, DK, 2, dtype=np.float32) / np.float32(DK))
    )
    freqs = pos.astype(np.float32)[:, :, None] * inv_freq[None, None, :]  # [B,S,32]
    cos = np.cos(freqs).astype(np.float32)
    sin = np.sin(freqs).astype(np.float32)
    jidx = (np.arange(P) % DK) // 2
    sign = np.where(np.arange(P) % 2 == 0, -1.0, 1.0).astype(np.float32)
    cos_tab = [np.ascontiguousarray(cos[b].T[jidx]).astype(BF) for b in range(B)]
    sin_tab = [
        np.ascontiguousarray(sin[b].T[jidx] * sign[:, None]).astype(BF)
        for b in range(B)
    ]

    masks = (np.arange(P)[:, None] <= np.arange(P)[None, :]).astype(BF)  # tril^T

    ident = np.eye(P, dtype=F16NP)

    # x residual split, DR layout
    xdr = []
    for b in range(B):
        xt = x[b].T.reshape(4, 2, P, S).transpose(2, 0, 1, 3)  # [128,4,2,S]
        xhi = _f8(xt)
        xlo = _f8(xt - xhi.astype(np.float32))
        xdr.append((np.ascontiguousarray(xhi), np.ascontiguousarray(xlo)))

    in_maps = []
    for c in range(N_CORES):
        b, hg = divmod(c, 4)
        heads = range(hg * HPC, (hg + 1) * HPC)
        q_rows = np.concatenate([W_qkv[h * DK:(h + 1) * DK] for h in heads])
        k_rows = np.concatenate(
            [W_qkv[D + h * DK:D + (h + 1) * DK] for h in heads]
        )
        v_rows = np.concatenate(
            [W_qkv[2 * D + h * DK:2 * D + (h + 1) * DK] for h in heads]
        )
        wqk = np.concatenate([q_rows, k_rows]) * WS  # [512, 1024]
        wv = v_rows * WS  # [256, 1024]
        wqk_t = _dr_major(wqk)
        wv_t = _dr_major(wv)
        wqkh = _f8(wqk_t)
        wqkl = _f8(wqk_t - wqkh.astype(np.float32))
        wvh = _f8(wv_t)
        wvl = _f8(wv_t - wvh.astype(np.float32))
        wo_sub = W_o[:, hg * 256:(hg + 1) * 256]  # [D, 256]
        wo = np.ascontiguousarray(
            wo_sub.T.reshape(2, P, D).transpose(1, 0, 2)
        ).astype(BF)
        in_maps.append(
            {
                "xh": xdr[b][0],
                "xl": xdr[b][1],
                "wqkh": np.ascontiguousarray(wqkh),
                "wqkl": np.ascontiguousarray(wqkl),
                "wvh": np.ascontiguousarray(wvh),
                "wvl": np.ascontiguousarray(wvl),
                "wo": wo,
                "cosv": cos_tab[b],
                "sinv": sin_tab[b],
                "masks": masks,
                "ident": ident,
            }
        )
    return in_maps


def _get_nc(reps=1, loop=False, probe=(), opts=None):
    key = f"nc{reps}_{loop}_{sorted(probe)}_{sorted((opts or {}).items())}"
    if key not in _CACHE:
        _CACHE[key] = _build_nc(reps, loop, probe, opts)
    return _CACHE[key]


def kernel(x, W_qkv, W_o, token_positions):
    nc = _get_nc()
    in_maps = _prep_in_maps(x, W_qkv, W_o, token_positions)
    res = run_bass_kernel_spmd(nc, in_maps, core_ids=list(range(N_CORES)))
    out = np.zeros((B, S, D), dtype=np.float32)
    for c in range(N_CORES):
        b = c // 4
        out[b] += np.asarray(res.results[c]["out_t"], dtype=np.float32).T
    return out


# revision 4
# speedup vs baseline: 1.0085x; 1.0085x over previous
"""Causal MHA with RoPE on 8 TRN2 NeuronCores — fp8/fp16 redesign.

Sharding: core c -> batch c//4, heads [4*(c%4), 4*(c%4)+4). Host sums the 4
partial output projections per batch.

Key structure vs the bf16 baseline:
- QKV projection in fp8e4m3 DoubleRow with unscaled residual splitting
  (x=xh+xl, W=wh+wl, three products accumulated in one PSUM group): 4x fewer
  PE rows than bf16 at ~bf16 accuracy.
- Scores in bf16 (q/k from RoPE), per-head 64-partition stationary tiles.
- Softmax probs in fp16. exp is split across engines: Activation runs true
  exp; DVE/Pool run a Schraudolph exp (tensor_scalar mult+add writing int16
  that bitcasts to fp16).
- PV transposed: stationary probs [keys, q128], moving V[keys, 65] with a
  WS-valued ones column producing the denominator per q-partition; normalize
  is a per-partition reciprocal+broadcast mul; PE transposes route the
  attention output into feature-major o2 for the bf16 output projection.
"""

import math
import sys

sys.path.insert(0, "/opt/trn_rl_repo")

import numpy as np
import ml_dtypes

import concourse.bass as bass
import concourse.bacc as bacc
import concourse.mybir as mybir
import concourse.tile as tile
from concourse.bass_utils import run_bass_kernel_spmd

B, S, D = 2, 2048, 1024
H, DK = 16, 64
THETA = 10000.0
HPC = 4
N_CORES = 8
P = 128
QTILE = 512
NQ = S // QTILE      # 4 phases
NKT = S // P         # 16 key tiles
NJ = S // 256        # 8 q256 tiles
BF = ml_dtypes.bfloat16
F8NP = ml_dtypes.float8_e4m3
F16NP = np.float16

WS = 32.0
ESCALE = 0.125 / (WS * WS)
A16 = ESCALE * 1024.0 / math.log(2.0)
B16 = 15360.5

_PAIRSWAP = [i + 1 if i % 2 == 0 else i - 1 for i in range(32)]

_CACHE = {}


def _build_nc(reps=1, loop=False, probe=(), opts=None):
    probe = set(probe)
    opts = dict(opts or {})
    prab_bufs = opts.get("prab_bufs", 36)
    pvlag = opts.get("pvlag", 5)
    rope_bufs = opts.get("rope_bufs", 6)
    ysb_bufs = opts.get("ysb_bufs", 6)
    exp_cycle = opts.get("exp_cycle", "aad")  # a=Act d=DVE per (j,kt); Pool
    exp_sched = opts.get(                     # cannot touch PSUM on TRN2
        "exp_sched", {0: "aaad", 1: "aaad", 2: "aadaad", 3: "aadadad"})
    ysb_cycle = opts.get("ysb_cycle", "ad")
    xcopy_eng = opts.get("xcopy_eng", "a")
    vcopy_eng = opts.get("vcopy_eng", "a")
    mask_eng = opts.get("mask_eng", "d")
    ropeadd_eng = opts.get("ropeadd_eng", "p")
    ropemul_eng = opts.get("ropemul_eng", "d")
    o2_eng = opts.get("o2_eng", "d")
    sc_bufs = opts.get("sc_bufs", 2)
    fill_order = opts.get("fill_order", "qo")
    mid_bufs = opts.get("mid_bufs", 2)
    po_bufs = opts.get("po_bufs", 2)
    f32 = mybir.dt.float32
    bf16 = mybir.dt.bfloat16
    fp16 = mybir.dt.float16
    fp8 = mybir.dt.float8e4
    i16 = mybir.dt.int16
    Exp = mybir.ActivationFunctionType.Exp
    DR = mybir.MatmulPerfMode.DoubleRow

    nc = bacc.Bacc()
    xh_d = nc.dram_tensor("xh", [P, 4, 2, S], fp8, kind="ExternalInput")
    xl_d = nc.dram_tensor("xl", [P, 4, 2, S], fp8, kind="ExternalInput")
    wqkh_d = nc.dram_tensor("wqkh", [P, 4, 2, 512], fp8, kind="ExternalInput")
    wqkl_d = nc.dram_tensor("wqkl", [P, 4, 2, 512], fp8, kind="ExternalInput")
    wvh_d = nc.dram_tensor("wvh", [P, 4, 2, 256], fp8, kind="ExternalInput")
    wvl_d = nc.dram_tensor("wvl", [P, 4, 2, 256], fp8, kind="ExternalInput")
    wo_d = nc.dram_tensor("wo", [P, 2, D], bf16, kind="ExternalInput")
    cos_d = nc.dram_tensor("cosv", [P, S], bf16, kind="ExternalInput")
    sin_d = nc.dram_tensor("sinv", [P, S], bf16, kind="ExternalInput")
    mask_d = nc.dram_tensor("masks", [P, P], bf16, kind="ExternalInput")
    ident_d = nc.dram_tensor("ident", [P, P], fp16, kind="ExternalInput")
    out_d = nc.dram_tensor("out_t", [D, S], fp16, kind="ExternalOutput")

    with tile.TileContext(nc) as tc:
        with (
            tc.tile_pool(name="const", bufs=1) as cp,
            tc.tile_pool(name="rope", bufs=rope_bufs) as ropep,
            tc.tile_pool(name="probs", bufs=prab_bufs) as probsp,
            tc.tile_pool(name="small", bufs=4) as smallp,
            tc.tile_pool(name="ysb", bufs=ysb_bufs) as ysbp,
            tc.tile_pool(name="ps_sc", bufs=sc_bufs, space="PSUM") as ps_sc,
            tc.tile_pool(name="ps_mid", bufs=mid_bufs, space="PSUM") as ps_mid,
            tc.tile_pool(name="ps_po", bufs=po_bufs, space="PSUM") as ps_po,
        ):
            xh = cp.tile([P, 4, 2, S], fp8, tag="xh")
            xl = cp.tile([P, 4, 2, S], fp8, tag="xl")
            wqkh = cp.tile([P, 4, 2, 512], fp8, tag="wqkh")
            wqkl = cp.tile([P, 4, 2, 512], fp8, tag="wqkl")
            wvh = cp.tile([P, 4, 2, 256], fp8, tag="wvh")
            wvl = cp.tile([P, 4, 2, 256], fp8, tag="wvl")
            wo = cp.tile([P, 2, D], bf16, tag="wo")
            cosv = cp.tile([P, S], bf16, tag="cos")
            sinv = cp.tile([P, S], bf16, tag="sin")
            maskt = cp.tile([P, P], bf16, tag="mask")
            ident = cp.tile([P, P], fp16, tag="ident")
            qsb = cp.tile([P, 2, S], bf16, tag="qsb")
            ksb = cp.tile([P, 2, S], bf16, tag="ksb")
            v2 = cp.tile([P, NKT, HPC, 65], fp16, tag="v2")
            o2 = cp.tile([P, 2, S], fp16, tag="o2")

            # Act table warmup during the input-DMA wait
            warm = smallp.tile([P, 2], f32, tag="warm", name="warm")
            nc.vector.memset(warm[:], 0.0)
            nc.scalar.activation(warm[:, 0:1], warm[:, 1:2],
                                 Exp, scale=1.0)

            # input DMAs: few big transfers, first QKV chain's first
            sl0 = slice(0, QTILE)
            for kk in range(2):
                ksl = slice(kk * 2, kk * 2 + 2)
                nc.sync.dma_start(wqkh[:, ksl, :, :], wqkh_d[:, ksl, :, :])
                nc.sync.dma_start(xh[:, ksl, :, sl0], xh_d[:, ksl, :, sl0])
            for kk in range(2):
                ksl = slice(kk * 2, kk * 2 + 2)
                nc.sync.dma_start(xl[:, ksl, :, sl0], xl_d[:, ksl, :, sl0])
            nc.sync.dma_start(wqkl[:], wqkl_d[:])
            nc.sync.dma_start(cosv[:, sl0], cos_d[:, sl0])
            nc.sync.dma_start(sinv[:, sl0], sin_d[:, sl0])
            nc.sync.dma_start(wvh[:], wvh_d[:])
            nc.sync.dma_start(wvl[:], wvl_d[:])
            for t in range(1, NQ):
                sl = slice(t * QTILE, (t + 1) * QTILE)
                nc.sync.dma_start(xh[:, :, :, sl], xh_d[:, :, :, sl])
                nc.sync.dma_start(xl[:, :, :, sl], xl_d[:, :, :, sl])
                nc.sync.dma_start(cosv[:, sl], cos_d[:, sl])
                nc.sync.dma_start(sinv[:, sl], sin_d[:, sl])
                if t == 1:
                    nc.sync.dma_start(maskt[:], mask_d[:])
                    nc.sync.dma_start(ident[:], ident_d[:])
            nc.sync.dma_start(wo[:], wo_d[:])
            nc.gpsimd.memset(v2[:], WS)

            exp_n = [0]
            ysb_n = [0]

            def qkv_dr(out_ap, stat_tile_pair, mov_tile_pair, stat_sl, mov_sl):
                """12 DoubleRow matmuls: 3 residual products x 4 ksteps."""
                prods = [
                    (stat_tile_pair[0], mov_tile_pair[0]),
                    (stat_tile_pair[0], mov_tile_pair[1]),
                    (stat_tile_pair[1], mov_tile_pair[0]),
                ]
                n = len(prods) * 4
                i = 0
                for st, mv in prods:
                    for ks in range(4):
                        nc.tensor.matmul(
                            out_ap,
                            st[(slice(None), ks) + stat_sl],
                            mv[(slice(None), ks) + mov_sl],
                            start=(i == 0),
                            stop=(i == n - 1),
                            perf_mode=DR,
                        )
                        i += 1

            def qkv_unit(t, mt):
                """one QKV mt-tile (q or k head-pair) + its rope chain"""
                nsl = slice(t * QTILE, (t + 1) * QTILE)
                pqk = ps_mid.tile([P, QTILE], f32, tag="m", name="pqk")
                if "noqkv" in probe:
                    nc.vector.memset(pqk[:], 0.01)
                else:
                    qkv_dr(
                        pqk[:],
                        (wqkh, wqkl),
                        (xh, xl),
                        (slice(None), slice(mt * 128, (mt + 1) * 128)),
                        (slice(None), nsl),
                    )
                # rope: copy on Act, shuffle+muls+add on DVE (bf16 2x)
                cpy = ropep.tile([P, QTILE], bf16, tag="cpy", name="cpy")
                if xcopy_eng == "a":
                    nc.scalar.copy(cpy[:], pqk[:])
                else:
                    nc.vector.tensor_copy(cpy[:], pqk[:])
                sw = ropep.tile([P, QTILE], bf16, tag="sw", name="sw")
                nc.vector.stream_shuffle(sw[:], cpy[:], _PAIRSWAP)
                t0 = ropep.tile([P, QTILE], bf16, tag="t0", name="t0")
                nc.vector.tensor_mul(t0[:], cpy[:], cosv[:, nsl])
                u0 = ropep.tile([P, QTILE], bf16, tag="u0", name="u0")
                rm = nc.gpsimd if ropemul_eng == "p" else nc.vector
                rm.tensor_mul(u0[:], sw[:], sinv[:, nsl])
                dst = qsb if mt < 2 else ksb
                radd = nc.gpsimd if ropeadd_eng == "p" else nc.vector
                radd.tensor_add(dst[:, mt % 2, nsl], t0[:], u0[:])

            def v_unit(t, tt):
                kt = t * 4 + tt
                pv = ps_mid.tile([P, 2, HPC, DK], f32, tag="m", name="pv")
                if "noqkv" in probe:
                    nc.vector.memset(pv[:, 0, :, :], 0.01)
                else:
                    qkv_dr(
                        pv[:, 0, :, :],
                        (xh, xl),
                        (wvh, wvl),
                        (slice(None), slice(kt * P, (kt + 1) * P)),
                        (slice(None), slice(None)),
                    )
                if vcopy_eng == "a":
                    nc.scalar.copy(v2[:, kt, :, 0:DK], pv[:, 0, :, :])
                else:
                    nc.vector.tensor_copy(v2[:, kt, :, 0:DK], pv[:, 0, :, :])

            prabs = {}

            def sc_unit(qt, kt, hp):
                """scores+exp(+mask) for one (kt, head-pair)"""
                if True:
                    ktsl = slice(kt * P, (kt + 1) * P)
                    r = kt - 4 * qt  # 0..3 diag band; negative for full kts
                    rq = max(r, 0) * 128
                    qsl = slice(qt * QTILE + rq, (qt + 1) * QTILE)
                    if True:
                        pst = ps_sc.tile([P, 2, QTILE], f32, tag="s", name="pst")
                        if "noscores" in probe:
                            nc.vector.memset(pst[:, :, rq:rq + 1], 0.0)
                        else:
                            for half in range(2):
                                psl = slice(half * 64, half * 64 + 64)
                                nc.tensor.matmul(
                                    pst[:, half, rq:],
                                    ksb[psl, hp, ktsl],
                                    qsb[psl, hp, qsl],
                                    start=True,
                                    stop=True,
                                    tile_position=(half * 64, 0),
                                )
                        prab = probsp.tile([P, 2, QTILE], fp16, tag="pr", name="pr")
                        cyc = exp_sched.get(qt, exp_cycle)
                        eng = cyc[exp_n[0] % len(cyc)]
                        exp_n[0] += 1
                        if "noexp" in probe:
                            nc.vector.memset(prab[:, :, rq:rq + 1], 0.001)
                        elif eng == "a":
                            nc.scalar.activation(
                                prab[:, :, rq:], pst[:, :, rq:], Exp, scale=ESCALE
                            )
                        else:
                            e = nc.vector if eng == "d" else nc.gpsimd
                            e.tensor_scalar(
                                prab[:, :, rq:].bitcast(i16),
                                pst[:, :, rq:],
                                A16,
                                B16,
                                op0=mybir.AluOpType.mult,
                                op1=mybir.AluOpType.add,
                            )
                        if r >= 0 and "nomask" not in probe:
                            meng = nc.gpsimd if mask_eng == "p" else nc.vector
                            meng.tensor_mul(
                                prab[:, :, rq:rq + P],
                                prab[:, :, rq:rq + P],
                                maskt[:, None, :].to_broadcast([P, 2, P]),
                            )
                        prabs[(kt, hp)] = prab

            def pv_unit(qt, qi):
                # PV + normalize for one q128; sequential per-(q128, head)
                # chains so each po bank has one open PSUM group at a time
                if True:
                    c = qi % 4
                    po = ps_po.tile([P, HPC, 128], f32, tag="po", name="po")
                    if "nopv" in probe:
                        nc.vector.memset(po[:], 1.0)
                    else:
                        for h in range(HPC):
                            for kt in range(qi + 1):
                                nc.tensor.matmul(
                                    po[:, h, 0:65],
                                    prabs[(kt, h // 2)][
                                        :, h % 2, c * 128:(c + 1) * 128
                                    ],
                                    v2[:, kt, h, :],
                                    start=(kt == 0),
                                    stop=(kt == qi),
                                )
                    if "nonorm" in probe:
                        nc.vector.memset(o2[:, :, qi * P:(qi + 1) * P], 0.01)
                        return
                    rcp = smallp.tile([P, HPC], f32, tag="rc", name="rcp")
                    nc.vector.reciprocal(rcp[:], po[:, :, 64:65])
                    onorm = smallp.tile([P, HPC, DK], fp16, tag="on", name="onorm")
                    nc.vector.tensor_mul(
                        onorm[:],
                        po[:, :, 0:DK],
                        rcp[:, :, None].to_broadcast([P, HPC, DK]),
                    )
                    if "notr" in probe:
                        nc.vector.memset(o2[:, :, qi * P:(qi + 1) * P], 0.01)
                        return
                    tr = ps_mid.tile([P, 2, 256], f32, tag="m", name="tr")
                    trh = tr[:].bitcast(fp16)  # [P, 2, 512] fp16 view
                    for kj in range(2):
                        nc.tensor.transpose(
                            trh[:, kj, 0:128],
                            onorm[:, 2 * kj:2 * kj + 2, :],
                            ident[:],
                        )
                    if o2_eng == "a":
                        nc.scalar.copy(
                            o2[:, :, qi * P:(qi + 1) * P], trh[:, :, 0:128])
                    else:
                        nc.vector.tensor_copy(
                            o2[:, :, qi * P:(qi + 1) * P], trh[:, :, 0:128])

            def oproj_unit(t, ot, qsl=None):
                qsl = qsl or slice(t * QTILE, (t + 1) * QTILE)
                w = qsl.stop - qsl.start
                py = ps_mid.tile([P, QTILE], f32, tag="m", name="py")
                for kj in range(2):
                    nc.tensor.matmul(
                        py[:, 0:w],
                        wo[:, kj, ot * 128:(ot + 1) * 128],
                        o2[:, kj, qsl],
                        start=(kj == 0),
                        stop=(kj == 1),
                    )
                if "noy" in probe:
                    return
                ysb = ysbp.tile([P, QTILE], fp16, tag="y", name="ysb")
                eng = ysb_cycle[ysb_n[0] % len(ysb_cycle)]
                ysb_n[0] += 1
                if eng == "a":
                    nc.scalar.copy(ysb[:, 0:w], py[:, 0:w])
                elif eng == "d":
                    nc.vector.tensor_copy(ysb[:, 0:w], py[:, 0:w])
                else:
                    nc.gpsimd.tensor_copy(ysb[:, 0:w], py[:, 0:w])
                nc.sync.dma_start(
                    out_d[ot * 128:(ot + 1) * 128, qsl], ysb[:, 0:w])

            def body():
                if "noattn" in probe:
                    nc.vector.memset(o2[:], 0.01)
                # phase 0 QKV up front
                for mt in range(4):
                    qkv_unit(0, mt)
                for tt in range(4):
                    v_unit(0, tt)
                for t in range(NQ):
                    # fill units: next phase's QKV/V, prev phase's oproj
                    fill = []
                    qkvf = []
                    if t + 1 < NQ:
                        qkvf += [(qkv_unit, (t + 1, mt)) for mt in range(4)]
                        qkvf += [(v_unit, (t + 1, tt)) for tt in range(4)]
                    opf = []
                    if t > 0:
                        opf += [(oproj_unit, (t - 1, ot)) for ot in range(8)]
                    if fill_order == "qo":
                        fill = qkvf + opf
                    elif fill_order == "oq":
                        fill = opf + qkvf
                    else:  # interleaved
                        n = max(len(qkvf), len(opf))
                        for i in range(n):
                            if i < len(opf):
                                fill.append(opf[i])
                            if i < len(qkvf):
                                fill.append(qkvf[i])
                    # interleave: scores stream + PV chains + fill units
                    nkt = 4 * (t + 1)
                    fi = 0
                    emitted_pv = 0
                    if "noattn" in probe:
                        seq = []
                    else:
                        seq = [(sc_unit, (t, kt, hp))
                               for kt in range(nkt) for hp in range(2)]
                    for n, (fn, args) in enumerate(seq):
                        fn(*args)
                        kt = args[1]
                        # a fill unit after every sc pair
                        if n % 2 == 1 and fi < len(fill):
                            fn2, a2 = fill[fi]
                            fn2(*a2)
                            fi += 1
                        # PV chain for qi once sc(kt=qi+pvlag) has been emitted
                        while (emitted_pv < 4
                               and 4 * t + emitted_pv + pvlag <= kt):
                            pv_unit(t, 4 * t + emitted_pv)
                            emitted_pv += 1
                    while emitted_pv < 4 and "noattn" not in probe:
                        pv_unit(t, 4 * t + emitted_pv)
                        emitted_pv += 1
                    while fi < len(fill):
                        fn2, a2 = fill[fi]
                        fn2(*a2)
                        fi += 1
                    prabs.clear()
                # last oproj
                for ot in range(8):
                    oproj_unit(NQ - 1, ot)

            if loop:
                with tc.For_i(0, reps, 1):
                    body()
            else:
                for _rep in range(reps):
                    body()
    nc.compile()
    return nc


def _f8(a):
    return np.asarray(a, dtype=F8NP)


def _dr_major(Wmat):
    """[M, 1024] -> [128, 4, 2, M]: (p, ks, i, m) = W[m, ks*256+i*128+p]."""
    M = Wmat.shape[0]
    return np.ascontiguousarray(
        Wmat.T.reshape(4, 2, P, M).transpose(2, 0, 1, 3)
    )


def _prep_in_maps(x, W_qkv, W_o, token_positions):
    x = np.asarray(x, dtype=np.float32)
    W_qkv = np.asarray(W_qkv, dtype=np.float32)
    W_o = np.asarray(W_o, dtype=np.float32)
    pos = np.asarray(token_positions)

    inv_freq = 1.0 / (
        np.float32(THETA) ** (np.arange(0, DK, 2, dtype=np.float32) / np.float32(DK))
    )
    freqs = pos.astype(np.float32)[:, :, None] * inv_freq[None, None, :]  # [B,S,32]
    cos = np.cos(freqs).astype(np.float32)
    sin = np.sin(freqs).astype(np.float32)
    jidx = (np.arange(P) % DK) // 2
    sign = np.where(np.arange(P) % 2 == 0, -1.0, 1.0).astype(np.float32)
    cos_tab = [np.ascontiguousarray(cos[b].T[jidx]).astype(BF) for b in range(B)]
    sin_tab = [
        np.ascontiguousarray(sin[b].T[jidx] * sign[:, None]).astype(BF)
        for b in range(B)
    ]

    masks = (np.arange(P)[:, None] <= np.arange(P)[None, :]).astype(BF)  # tril^T

    ident = np.eye(P, dtype=F16NP)

    # x residual split, DR layout
    xdr = []
    for b in range(B):
        xt = x[b].T.reshape(4, 2, P, S).transpose(2, 0, 1, 3)  # [128,4,2,S]
        xhi = _f8(xt)
        xlo = _f8(xt - xhi.astype(np.float32))
        xdr.append((np.ascontiguousarray(xhi), np.ascontiguousarray(xlo)))

    in_maps = []
    for c in range(N_CORES):
        b, hg = divmod(c, 4)
        heads = range(hg * HPC, (hg + 1) * HPC)
        q_rows = np.concatenate([W_qkv[h * DK:(h + 1) * DK] for h in heads])
        k_rows = np.concatenate(
            [W_qkv[D + h * DK:D + (h + 1) * DK] for h in heads]
        )
        v_rows = np.concatenate(
            [W_qkv[2 * D + h * DK:2 * D + (h + 1) * DK] for h in heads]
        )
        wqk = np.concatenate([q_rows, k_rows]) * WS  # [512, 1024]
        wv = v_rows * WS  # [256, 1024]
        wqk_t = _dr_major(wqk)
        wv_t = _dr_major(wv)
        wqkh = _f8(wqk_t)
        wqkl = _f8(wqk_t - wqkh.astype(np.float32))
        wvh = _f8(wv_t)
        wvl = _f8(wv_t - wvh.astype(np.float32))
        wo_sub = W_o[:, hg * 256:(hg + 1) * 256]  # [D, 256]
        wo = np.ascontiguousarray(
            wo_sub.T.reshape(2, P, D).transpose(1, 0, 2)
        ).astype(BF)
        in_maps.append(
            {
                "xh": xdr[b][0],
                "xl": xdr[b][1],
                "wqkh": np.ascontiguousarray(wqkh),
                "wqkl": np.ascontiguousarray(wqkl),
                "wvh": np.ascontiguousarray(wvh),
                "wvl": np.ascontiguousarray(wvl),
                "wo": wo,
                "cosv": cos_tab[b],
                "sinv": sin_tab[b],
                "masks": masks,
                "ident": ident,
            }
        )
    return in_maps


def _get_nc(reps=1, loop=False, probe=(), opts=None):
    key = f"nc{reps}_{loop}_{sorted(probe)}_{sorted((opts or {}).items())}"
    if key not in _CACHE:
        _CACHE[key] = _build_nc(reps, loop, probe, opts)
    return _CACHE[key]


def kernel(x, W_qkv, W_o, token_positions):
    nc = _get_nc()
    in_maps = _prep_in_maps(x, W_qkv, W_o, token_positions)
    res = run_bass_kernel_spmd(nc, in_maps, core_ids=list(range(N_CORES)))
    out = np.zeros((B, S, D), dtype=np.float32)
    for c in range(N_CORES):
        b = c // 4
        out[b] += np.asarray(res.results[c]["out_t"], dtype=np.float32).T
    return out


# revision 5
# speedup vs baseline: 1.0239x; 1.0153x over previous
"""Causal MHA with RoPE on 8 TRN2 NeuronCores — fp8/fp16 redesign.

Sharding: core c -> batch c//4, heads [4*(c%4), 4*(c%4)+4). Host sums the 4
partial output projections per batch.

Key structure vs the bf16 baseline:
- QKV projection in fp8e4m3 DoubleRow with unscaled residual splitting
  (x=xh+xl, W=wh+wl, three products accumulated in one PSUM group): 4x fewer
  PE rows than bf16 at ~bf16 accuracy.
- Scores in bf16 (q/k from RoPE), per-head 64-partition stationary tiles.
- Softmax probs in fp16. exp is split across engines: Activation runs true
  exp; DVE/Pool run a Schraudolph exp (tensor_scalar mult+add writing int16
  that bitcasts to fp16).
- PV transposed: stationary probs [keys, q128], moving V[keys, 65] with a
  WS-valued ones column producing the denominator per q-partition; normalize
  is a per-partition reciprocal+broadcast mul; PE transposes route the
  attention output into feature-major o2 for the bf16 output projection.
"""

import math
import sys

sys.path.insert(0, "/opt/trn_rl_repo")

import numpy as np
import ml_dtypes

import concourse.bass as bass
import concourse.bacc as bacc
import concourse.mybir as mybir
import concourse.tile as tile
from concourse.bass_utils import run_bass_kernel_spmd

B, S, D = 2, 2048, 1024
H, DK = 16, 64
THETA = 10000.0
HPC = 4
N_CORES = 8
P = 128
QTILE = 512
NQ = S // QTILE      # 4 phases
NKT = S // P         # 16 key tiles
NJ = S // 256        # 8 q256 tiles
BF = ml_dtypes.bfloat16
F8NP = ml_dtypes.float8_e4m3
F16NP = np.float16

WS = 32.0
ESCALE = 0.125 / (WS * WS)
A16 = ESCALE * 1024.0 / math.log(2.0)
B16 = 15360.5

_PAIRSWAP = [i + 1 if i % 2 == 0 else i - 1 for i in range(32)]

_CACHE = {}


def _build_nc(reps=1, loop=False, probe=(), opts=None):
    probe = set(probe)
    opts = dict(opts or {})
    prab_bufs = opts.get("prab_bufs", 36)
    pvlag = opts.get("pvlag", 5)
    rope_bufs = opts.get("rope_bufs", 6)
    ysb_bufs = opts.get("ysb_bufs", 6)
    exp_cycle = opts.get("exp_cycle", "aad")  # a=Act d=DVE per (j,kt); Pool
    exp_sched = opts.get(                     # cannot touch PSUM on TRN2
        "exp_sched", {0: "aaad", 1: "aaad", 2: "aadaad", 3: "aadadad"})
    ysb_cycle = opts.get("ysb_cycle", "ad")
    xcopy_eng = opts.get("xcopy_eng", "a")
    vcopy_eng = opts.get("vcopy_eng", "a")
    mask_eng = opts.get("mask_eng", "d")
    ropeadd_eng = opts.get("ropeadd_eng", "p")
    ropemul_eng = opts.get("ropemul_eng", "d")
    o2_eng = opts.get("o2_eng", "d")
    sc_bufs = opts.get("sc_bufs", 2)
    fill_order = opts.get("fill_order", "qo")
    tail_order = opts.get("tail_order", "fp")
    diag_eng = opts.get("diag_eng", None)
    mid_bufs = opts.get("mid_bufs", 2)
    po_bufs = opts.get("po_bufs", 2)
    f32 = mybir.dt.float32
    bf16 = mybir.dt.bfloat16
    fp16 = mybir.dt.float16
    fp8 = mybir.dt.float8e4
    i16 = mybir.dt.int16
    Exp = mybir.ActivationFunctionType.Exp
    DR = mybir.MatmulPerfMode.DoubleRow

    nc = bacc.Bacc()
    xh_d = nc.dram_tensor("xh", [P, 4, 2, S], fp8, kind="ExternalInput")
    xl_d = nc.dram_tensor("xl", [P, 4, 2, S], fp8, kind="ExternalInput")
    wqkh_d = nc.dram_tensor("wqkh", [P, 4, 2, 512], fp8, kind="ExternalInput")
    wqkl_d = nc.dram_tensor("wqkl", [P, 4, 2, 512], fp8, kind="ExternalInput")
    wvh_d = nc.dram_tensor("wvh", [P, 4, 2, 256], fp8, kind="ExternalInput")
    wvl_d = nc.dram_tensor("wvl", [P, 4, 2, 256], fp8, kind="ExternalInput")
    wo_d = nc.dram_tensor("wo", [P, 2, D], bf16, kind="ExternalInput")
    cos_d = nc.dram_tensor("cosv", [P, S], bf16, kind="ExternalInput")
    sin_d = nc.dram_tensor("sinv", [P, S], bf16, kind="ExternalInput")
    mask_d = nc.dram_tensor("masks", [P, P], bf16, kind="ExternalInput")
    ident_d = nc.dram_tensor("ident", [P, P], fp16, kind="ExternalInput")
    out_d = nc.dram_tensor("out_t", [D, S], fp16, kind="ExternalOutput")

    with tile.TileContext(nc) as tc:
        with (
            tc.tile_pool(name="const", bufs=1) as cp,
            tc.tile_pool(name="rope", bufs=rope_bufs) as ropep,
            tc.tile_pool(name="probs", bufs=prab_bufs) as probsp,
            tc.tile_pool(name="small", bufs=4) as smallp,
            tc.tile_pool(name="ysb", bufs=ysb_bufs) as ysbp,
            tc.tile_pool(name="ps_sc", bufs=sc_bufs, space="PSUM") as ps_sc,
            tc.tile_pool(name="ps_mid", bufs=mid_bufs, space="PSUM") as ps_mid,
            tc.tile_pool(name="ps_po", bufs=po_bufs, space="PSUM") as ps_po,
        ):
            xh = cp.tile([P, 4, 2, S], fp8, tag="xh")
            xl = cp.tile([P, 4, 2, S], fp8, tag="xl")
            wqkh = cp.tile([P, 4, 2, 512], fp8, tag="wqkh")
            wqkl = cp.tile([P, 4, 2, 512], fp8, tag="wqkl")
            wvh = cp.tile([P, 4, 2, 256], fp8, tag="wvh")
            wvl = cp.tile([P, 4, 2, 256], fp8, tag="wvl")
            wo = cp.tile([P, 2, D], bf16, tag="wo")
            cosv = cp.tile([P, S], bf16, tag="cos")
            sinv = cp.tile([P, S], bf16, tag="sin")
            maskt = cp.tile([P, P], bf16, tag="mask")
            ident = cp.tile([P, P], fp16, tag="ident")
            qsb = cp.tile([P, 2, S], bf16, tag="qsb")
            ksb = cp.tile([P, 2, S], bf16, tag="ksb")
            v2 = cp.tile([P, NKT, HPC, 65], fp16, tag="v2")
            o2 = cp.tile([P, 2, S], fp16, tag="o2")

            # Act table warmup during the input-DMA wait
            warm = smallp.tile([P, 2], f32, tag="warm", name="warm")
            nc.vector.memset(warm[:], 0.0)
            nc.scalar.activation(warm[:, 0:1], warm[:, 1:2],
                                 Exp, scale=1.0)

            # input DMAs: few big transfers, first QKV chain's first
            sl0 = slice(0, QTILE)
            for kk in range(2):
                ksl = slice(kk * 2, kk * 2 + 2)
                nc.sync.dma_start(wqkh[:, ksl, :, :], wqkh_d[:, ksl, :, :])
                nc.sync.dma_start(xh[:, ksl, :, sl0], xh_d[:, ksl, :, sl0])
            for kk in range(2):
                ksl = slice(kk * 2, kk * 2 + 2)
                nc.sync.dma_start(xl[:, ksl, :, sl0], xl_d[:, ksl, :, sl0])
            nc.sync.dma_start(wqkl[:], wqkl_d[:])
            nc.sync.dma_start(cosv[:, sl0], cos_d[:, sl0])
            nc.sync.dma_start(sinv[:, sl0], sin_d[:, sl0])
            nc.sync.dma_start(wvh[:], wvh_d[:])
            nc.sync.dma_start(wvl[:], wvl_d[:])
            for t in range(1, NQ):
                sl = slice(t * QTILE, (t + 1) * QTILE)
                nc.sync.dma_start(xh[:, :, :, sl], xh_d[:, :, :, sl])
                nc.sync.dma_start(xl[:, :, :, sl], xl_d[:, :, :, sl])
                nc.sync.dma_start(cosv[:, sl], cos_d[:, sl])
                nc.sync.dma_start(sinv[:, sl], sin_d[:, sl])
                if t == 1:
                    nc.sync.dma_start(maskt[:], mask_d[:])
                    nc.sync.dma_start(ident[:], ident_d[:])
            nc.sync.dma_start(wo[:], wo_d[:])
            nc.gpsimd.memset(v2[:], WS)

            exp_n = [0]
            ysb_n = [0]

            def qkv_dr(out_ap, stat_tile_pair, mov_tile_pair, stat_sl, mov_sl):
                """12 DoubleRow matmuls: 3 residual products x 4 ksteps."""
                prods = [
                    (stat_tile_pair[0], mov_tile_pair[0]),
                    (stat_tile_pair[0], mov_tile_pair[1]),
                    (stat_tile_pair[1], mov_tile_pair[0]),
                ]
                n = len(prods) * 4
                i = 0
                for st, mv in prods:
                    for ks in range(4):
                        nc.tensor.matmul(
                            out_ap,
                            st[(slice(None), ks) + stat_sl],
                            mv[(slice(None), ks) + mov_sl],
                            start=(i == 0),
                            stop=(i == n - 1),
                            perf_mode=DR,
                        )
                        i += 1

            def qkv_unit(t, mt):
                """one QKV mt-tile (q or k head-pair) + its rope chain"""
                nsl = slice(t * QTILE, (t + 1) * QTILE)
                pqk = ps_mid.tile([P, QTILE], f32, tag="m", name="pqk")
                if "noqkv" in probe:
                    nc.vector.memset(pqk[:], 0.01)
                else:
                    qkv_dr(
                        pqk[:],
                        (wqkh, wqkl),
                        (xh, xl),
                        (slice(None), slice(mt * 128, (mt + 1) * 128)),
                        (slice(None), nsl),
                    )
                # rope: copy on Act, shuffle+muls+add on DVE (bf16 2x)
                cpy = ropep.tile([P, QTILE], bf16, tag="cpy", name="cpy")
                if xcopy_eng == "a":
                    nc.scalar.copy(cpy[:], pqk[:])
                else:
                    nc.vector.tensor_copy(cpy[:], pqk[:])
                sw = ropep.tile([P, QTILE], bf16, tag="sw", name="sw")
                nc.vector.stream_shuffle(sw[:], cpy[:], _PAIRSWAP)
                t0 = ropep.tile([P, QTILE], bf16, tag="t0", name="t0")
                nc.vector.tensor_mul(t0[:], cpy[:], cosv[:, nsl])
                u0 = ropep.tile([P, QTILE], bf16, tag="u0", name="u0")
                rm = nc.gpsimd if ropemul_eng == "p" else nc.vector
                rm.tensor_mul(u0[:], sw[:], sinv[:, nsl])
                dst = qsb if mt < 2 else ksb
                radd = nc.gpsimd if ropeadd_eng == "p" else nc.vector
                radd.tensor_add(dst[:, mt % 2, nsl], t0[:], u0[:])

            def v_unit(t, tt):
                kt = t * 4 + tt
                pv = ps_mid.tile([P, 2, HPC, DK], f32, tag="m", name="pv")
                if "noqkv" in probe:
                    nc.vector.memset(pv[:, 0, :, :], 0.01)
                else:
                    qkv_dr(
                        pv[:, 0, :, :],
                        (xh, xl),
                        (wvh, wvl),
                        (slice(None), slice(kt * P, (kt + 1) * P)),
                        (slice(None), slice(None)),
                    )
                if vcopy_eng == "a":
                    nc.scalar.copy(v2[:, kt, :, 0:DK], pv[:, 0, :, :])
                else:
                    nc.vector.tensor_copy(v2[:, kt, :, 0:DK], pv[:, 0, :, :])

            prabs = {}

            def sc_unit(qt, kt, hp):
                """scores+exp(+mask) for one (kt, head-pair)"""
                if True:
                    ktsl = slice(kt * P, (kt + 1) * P)
                    r = kt - 4 * qt  # 0..3 diag band; negative for full kts
                    rq = max(r, 0) * 128
                    qsl = slice(qt * QTILE + rq, (qt + 1) * QTILE)
                    if True:
                        pst = ps_sc.tile([P, 2, QTILE], f32, tag="s", name="pst")
                        if "noscores" in probe:
                            nc.vector.memset(pst[:, :, rq:rq + 1], 0.0)
                        else:
                            for half in range(2):
                                psl = slice(half * 64, half * 64 + 64)
                                nc.tensor.matmul(
                                    pst[:, half, rq:],
                                    ksb[psl, hp, ktsl],
                                    qsb[psl, hp, qsl],
                                    start=True,
                                    stop=True,
                                    tile_position=(half * 64, 0),
                                )
                        prab = probsp.tile([P, 2, QTILE], fp16, tag="pr", name="pr")
                        if diag_eng and r >= 0:
                            eng = diag_eng
                        else:
                            cyc = exp_sched.get(qt, exp_cycle)
                            eng = cyc[exp_n[0] % len(cyc)]
                            exp_n[0] += 1
                        if "noexp" in probe:
                            nc.vector.memset(prab[:, :, rq:rq + 1], 0.001)
                        elif eng == "a":
                            nc.scalar.activation(
                                prab[:, :, rq:], pst[:, :, rq:], Exp, scale=ESCALE
                            )
                        else:
                            e = nc.vector if eng == "d" else nc.gpsimd
                            e.tensor_scalar(
                                prab[:, :, rq:].bitcast(i16),
                                pst[:, :, rq:],
                                A16,
                                B16,
                                op0=mybir.AluOpType.mult,
                                op1=mybir.AluOpType.add,
                            )
                        if r >= 0 and "nomask" not in probe:
                            meng = nc.gpsimd if mask_eng == "p" else nc.vector
                            meng.tensor_mul(
                                prab[:, :, rq:rq + P],
                                prab[:, :, rq:rq + P],
                                maskt[:, None, :].to_broadcast([P, 2, P]),
                            )
                        prabs[(kt, hp)] = prab

            def pv_unit(qt, qi):
                # PV + normalize for one q128; sequential per-(q128, head)
                # chains so each po bank has one open PSUM group at a time
                if True:
                    c = qi % 4
                    po = ps_po.tile([P, HPC, 128], f32, tag="po", name="po")
                    if "nopv" in probe:
                        nc.vector.memset(po[:], 1.0)
                    else:
                        for h in range(HPC):
                            for kt in range(qi + 1):
                                nc.tensor.matmul(
                                    po[:, h, 0:65],
                                    prabs[(kt, h // 2)][
                                        :, h % 2, c * 128:(c + 1) * 128
                                    ],
                                    v2[:, kt, h, :],
                                    start=(kt == 0),
                                    stop=(kt == qi),
                                )
                    if "nonorm" in probe:
                        nc.vector.memset(o2[:, :, qi * P:(qi + 1) * P], 0.01)
                        return
                    rcp = smallp.tile([P, HPC], f32, tag="rc", name="rcp")
                    nc.vector.reciprocal(rcp[:], po[:, :, 64:65])
                    onorm = smallp.tile([P, HPC, DK], fp16, tag="on", name="onorm")
                    nc.vector.tensor_mul(
                        onorm[:],
                        po[:, :, 0:DK],
                        rcp[:, :, None].to_broadcast([P, HPC, DK]),
                    )
                    if "notr" in probe:
                        nc.vector.memset(o2[:, :, qi * P:(qi + 1) * P], 0.01)
                        return
                    tr = ps_mid.tile([P, 2, 256], f32, tag="m", name="tr")
                    trh = tr[:].bitcast(fp16)  # [P, 2, 512] fp16 view
                    for kj in range(2):
                        nc.tensor.transpose(
                            trh[:, kj, 0:128],
                            onorm[:, 2 * kj:2 * kj + 2, :],
                            ident[:],
                        )
                    if o2_eng == "a":
                        nc.scalar.copy(
                            o2[:, :, qi * P:(qi + 1) * P], trh[:, :, 0:128])
                    else:
                        nc.vector.tensor_copy(
                            o2[:, :, qi * P:(qi + 1) * P], trh[:, :, 0:128])

            def oproj_unit(t, ot, qsl=None):
                qsl = qsl or slice(t * QTILE, (t + 1) * QTILE)
                w = qsl.stop - qsl.start
                py = ps_mid.tile([P, QTILE], f32, tag="m", name="py")
                for kj in range(2):
                    nc.tensor.matmul(
                        py[:, 0:w],
                        wo[:, kj, ot * 128:(ot + 1) * 128],
                        o2[:, kj, qsl],
                        start=(kj == 0),
                        stop=(kj == 1),
                    )
                if "noy" in probe:
                    return
                ysb = ysbp.tile([P, QTILE], fp16, tag="y", name="ysb")
                eng = ysb_cycle[ysb_n[0] % len(ysb_cycle)]
                ysb_n[0] += 1
                if eng == "a":
                    nc.scalar.copy(ysb[:, 0:w], py[:, 0:w])
                elif eng == "d":
                    nc.vector.tensor_copy(ysb[:, 0:w], py[:, 0:w])
                else:
                    nc.gpsimd.tensor_copy(ysb[:, 0:w], py[:, 0:w])
                nc.sync.dma_start(
                    out_d[ot * 128:(ot + 1) * 128, qsl], ysb[:, 0:w])

            def body():
                if "noattn" in probe:
                    nc.vector.memset(o2[:], 0.01)
                # phase 0 QKV up front
                for mt in range(4):
                    qkv_unit(0, mt)
                for tt in range(4):
                    v_unit(0, tt)
                for t in range(NQ):
                    # fill units: next phase's QKV/V, prev phase's oproj
                    fill = []
                    qkvf = []
                    if t + 1 < NQ:
                        qkvf += [(qkv_unit, (t + 1, mt)) for mt in range(4)]
                        qkvf += [(v_unit, (t + 1, tt)) for tt in range(4)]
                    opf = []
                    if t > 0:
                        opf += [(oproj_unit, (t - 1, ot)) for ot in range(8)]
                    if fill_order == "qo":
                        fill = qkvf + opf
                    elif fill_order == "oq":
                        fill = opf + qkvf
                    else:  # interleaved
                        n = max(len(qkvf), len(opf))
                        for i in range(n):
                            if i < len(opf):
                                fill.append(opf[i])
                            if i < len(qkvf):
                                fill.append(qkvf[i])
                    # interleave: scores stream + PV chains + fill units
                    nkt = 4 * (t + 1)
                    fi = 0
                    emitted_pv = 0
                    if "noattn" in probe:
                        seq = []
                    else:
                        seq = [(sc_unit, (t, kt, hp))
                               for kt in range(nkt) for hp in range(2)]
                    for n, (fn, args) in enumerate(seq):
                        fn(*args)
                        kt = args[1]
                        # a fill unit after every sc pair
                        if n % 2 == 1 and fi < len(fill):
                            fn2, a2 = fill[fi]
                            fn2(*a2)
                            fi += 1
                        # PV chain for qi once sc(kt=qi+pvlag) has been emitted
                        while (emitted_pv < 4
                               and 4 * t + emitted_pv + pvlag <= kt):
                            pv_unit(t, 4 * t + emitted_pv)
                            emitted_pv += 1
                    if tail_order == "fp":
                        while fi < len(fill):
                            fn2, a2 = fill[fi]
                            fn2(*a2)
                            fi += 1
                    while emitted_pv < 4 and "noattn" not in probe:
                        pv_unit(t, 4 * t + emitted_pv)
                        emitted_pv += 1
                        if tail_order == "alt" and fi < len(fill):
                            fn2, a2 = fill[fi]
                            fn2(*a2)
                            fi += 1
                    while fi < len(fill):
                        fn2, a2 = fill[fi]
                        fn2(*a2)
                        fi += 1
                    prabs.clear()
                # last oproj
                for ot in range(8):
                    oproj_unit(NQ - 1, ot)

            if loop:
                with tc.For_i(0, reps, 1):
                    body()
            else:
                for _rep in range(reps):
                    body()
    nc.compile()
    return nc


def _f8(a):
    return np.asarray(a, dtype=F8NP)


def _dr_major(Wmat):
    """[M, 1024] -> [128, 4, 2, M]: (p, ks, i, m) = W[m, ks*256+i*128+p]."""
    M = Wmat.shape[0]
    return np.ascontiguousarray(
        Wmat.T.reshape(4, 2, P, M).transpose(2, 0, 1, 3)
    )


def _prep_in_maps(x, W_qkv, W_o, token_positions):
    x = np.asarray(x, dtype=np.float32)
    W_qkv = np.asarray(W_qkv, dtype=np.float32)
    W_o = np.asarray(W_o, dtype=np.float32)
    pos = np.asarray(token_positions)

    inv_freq = 1.0 / (
        np.float32(THETA) ** (np.arange(0, DK, 2, dtype=np.float32) / np.float32(DK))
    )
    freqs = pos.astype(np.float32)[:, :, None] * inv_freq[None, None, :]  # [B,S,32]
    cos = np.cos(freqs).astype(np.float32)
    sin = np.sin(freqs).astype(np.float32)
    jidx = (np.arange(P) % DK) // 2
    sign = np.where(np.arange(P) % 2 == 0, -1.0, 1.0).astype(np.float32)
    cos_tab = [np.ascontiguousarray(cos[b].T[jidx]).astype(BF) for b in range(B)]
    sin_tab = [
        np.ascontiguousarray(sin[b].T[jidx] * sign[:, None]).astype(BF)
        for b in range(B)
    ]

    masks = (np.arange(P)[:, None] <= np.arange(P)[None, :]).astype(BF)  # tril^T

    ident = np.eye(P, dtype=F16NP)

    # x residual split, DR layout
    xdr = []
    for b in range(B):
        xt = x[b].T.reshape(4, 2, P, S).transpose(2, 0, 1, 3)  # [128,4,2,S]
        xhi = _f8(xt)
        xlo = _f8(xt - xhi.astype(np.float32))
        xdr.append((np.ascontiguousarray(xhi), np.ascontiguousarray(xlo)))

    in_maps = []
    for c in range(N_CORES):
        b, hg = divmod(c, 4)
        heads = range(hg * HPC, (hg + 1) * HPC)
        q_rows = np.concatenate([W_qkv[h * DK:(h + 1) * DK] for h in heads])
        k_rows = np.concatenate(
            [W_qkv[D + h * DK:D + (h + 1) * DK] for h in heads]
        )
        v_rows = np.concatenate(
            [W_qkv[2 * D + h * DK:2 * D + (h + 1) * DK] for h in heads]
        )
        wqk = np.concatenate([q_rows, k_rows]) * WS  # [512, 1024]
        wv = v_rows * WS  # [256, 1024]
        wqk_t = _dr_major(wqk)
        wv_t = _dr_major(wv)
        wqkh = _f8(wqk_t)
        wqkl = _f8(wqk_t - wqkh.astype(np.float32))
        wvh = _f8(wv_t)
        wvl = _f8(wv_t - wvh.astype(np.float32))
        wo_sub = W_o[:, hg * 256:(hg + 1) * 256]  # [D, 256]
        wo = np.ascontiguousarray(
            wo_sub.T.reshape(2, P, D).transpose(1, 0, 2)
        ).astype(BF)
        in_maps.append(
            {
                "xh": xdr[b][0],
                "xl": xdr[b][1],
                "wqkh": np.ascontiguousarray(wqkh),
                "wqkl": np.ascontiguousarray(wqkl),
                "wvh": np.ascontiguousarray(wvh),
                "wvl": np.ascontiguousarray(wvl),
                "wo": wo,
                "cosv": cos_tab[b],
                "sinv": sin_tab[b],
                "masks": masks,
                "ident": ident,
            }
        )
    return in_maps


def _get_nc(reps=1, loop=False, probe=(), opts=None):
    key = f"nc{reps}_{loop}_{sorted(probe)}_{sorted((opts or {}).items())}"
    if key not in _CACHE:
        _CACHE[key] = _build_nc(reps, loop, probe, opts)
    return _CACHE[key]


def kernel(x, W_qkv, W_o, token_positions):
    nc = _get_nc()
    in_maps = _prep_in_maps(x, W_qkv, W_o, token_positions)
    res = run_bass_kernel_spmd(nc, in_maps, core_ids=list(range(N_CORES)))
    out = np.zeros((B, S, D), dtype=np.float32)
    for c in range(N_CORES):
        b = c // 4
        out[b] += np.asarray(res.results[c]["out_t"], dtype=np.float32).T
    return out


# revision 6
# speedup vs baseline: 1.0255x; 1.0015x over previous
"""Causal MHA with RoPE on 8 TRN2 NeuronCores — fp8/fp16 redesign.

Sharding: core c -> batch c//4, heads [4*(c%4), 4*(c%4)+4). Host sums the 4
partial output projections per batch.

Key structure vs the bf16 baseline:
- QKV projection in fp8e4m3 DoubleRow with unscaled residual splitting
  (x=xh+xl, W=wh+wl, three products accumulated in one PSUM group): 4x fewer
  PE rows than bf16 at ~bf16 accuracy.
- Scores in bf16 (q/k from RoPE), per-head 64-partition stationary tiles.
- Softmax probs in fp16. exp is split across engines: Activation runs true
  exp; DVE/Pool run a Schraudolph exp (tensor_scalar mult+add writing int16
  that bitcasts to fp16).
- PV transposed: stationary probs [keys, q128], moving V[keys, 65] with a
  WS-valued ones column producing the denominator per q-partition; normalize
  is a per-partition reciprocal+broadcast mul; PE transposes route the
  attention output into feature-major o2 for the bf16 output projection.
"""

import math
import sys

sys.path.insert(0, "/opt/trn_rl_repo")

import numpy as np
import ml_dtypes

import concourse.bass as bass
import concourse.bacc as bacc
import concourse.mybir as mybir
import concourse.tile as tile
from concourse.bass_utils import run_bass_kernel_spmd

B, S, D = 2, 2048, 1024
H, DK = 16, 64
THETA = 10000.0
HPC = 4
N_CORES = 8
P = 128
QTILE = 512
NQ = S // QTILE      # 4 phases
NKT = S // P         # 16 key tiles
NJ = S // 256        # 8 q256 tiles
BF = ml_dtypes.bfloat16
F8NP = ml_dtypes.float8_e4m3
F16NP = np.float16

WS = 32.0
ESCALE = 0.125 / (WS * WS)
A16 = ESCALE * 1024.0 / math.log(2.0)
B16 = 15360.5

_PAIRSWAP = [i + 1 if i % 2 == 0 else i - 1 for i in range(32)]

_CACHE = {}


def _build_nc(reps=1, loop=False, probe=(), opts=None):
    probe = set(probe)
    opts = dict(opts or {})
    prab_bufs = opts.get("prab_bufs", 36)
    pvlag = opts.get("pvlag", 5)
    rope_bufs = opts.get("rope_bufs", 6)
    ysb_bufs = opts.get("ysb_bufs", 6)
    exp_cycle = opts.get("exp_cycle", "aad")  # a=Act d=DVE per (j,kt); Pool
    exp_sched = opts.get(                     # cannot touch PSUM on TRN2
        "exp_sched", {0: "aaad", 1: "aad", 2: "aadad", 3: "aadadad"})
    ysb_cycle = opts.get("ysb_cycle", "ad")
    xcopy_eng = opts.get("xcopy_eng", "a")
    vcopy_eng = opts.get("vcopy_eng", "a")
    mask_eng = opts.get("mask_eng", "d")
    ropeadd_eng = opts.get("ropeadd_eng", "p")
    ropemul_eng = opts.get("ropemul_eng", "d")
    o2_eng = opts.get("o2_eng", "d")
    sc_bufs = opts.get("sc_bufs", 2)
    fill_order = opts.get("fill_order", "qo")
    tail_order = opts.get("tail_order", "fp")
    diag_eng = opts.get("diag_eng", None)
    mid_bufs = opts.get("mid_bufs", 2)
    po_bufs = opts.get("po_bufs", 2)
    f32 = mybir.dt.float32
    bf16 = mybir.dt.bfloat16
    fp16 = mybir.dt.float16
    fp8 = mybir.dt.float8e4
    i16 = mybir.dt.int16
    Exp = mybir.ActivationFunctionType.Exp
    DR = mybir.MatmulPerfMode.DoubleRow

    nc = bacc.Bacc()
    xh_d = nc.dram_tensor("xh", [P, 4, 2, S], fp8, kind="ExternalInput")
    xl_d = nc.dram_tensor("xl", [P, 4, 2, S], fp8, kind="ExternalInput")
    wqkh_d = nc.dram_tensor("wqkh", [P, 4, 2, 512], fp8, kind="ExternalInput")
    wqkl_d = nc.dram_tensor("wqkl", [P, 4, 2, 512], fp8, kind="ExternalInput")
    wvh_d = nc.dram_tensor("wvh", [P, 4, 2, 256], fp8, kind="ExternalInput")
    wvl_d = nc.dram_tensor("wvl", [P, 4, 2, 256], fp8, kind="ExternalInput")
    wo_d = nc.dram_tensor("wo", [P, 2, D], bf16, kind="ExternalInput")
    cos_d = nc.dram_tensor("cosv", [P, S], bf16, kind="ExternalInput")
    sin_d = nc.dram_tensor("sinv", [P, S], bf16, kind="ExternalInput")
    mask_d = nc.dram_tensor("masks", [P, P], bf16, kind="ExternalInput")
    ident_d = nc.dram_tensor("ident", [P, P], fp16, kind="ExternalInput")
    out_d = nc.dram_tensor("out_t", [D, S], fp16, kind="ExternalOutput")

    with tile.TileContext(nc) as tc:
        with (
            tc.tile_pool(name="const", bufs=1) as cp,
            tc.tile_pool(name="rope", bufs=rope_bufs) as ropep,
            tc.tile_pool(name="probs", bufs=prab_bufs) as probsp,
            tc.tile_pool(name="small", bufs=4) as smallp,
            tc.tile_pool(name="ysb", bufs=ysb_bufs) as ysbp,
            tc.tile_pool(name="ps_sc", bufs=sc_bufs, space="PSUM") as ps_sc,
            tc.tile_pool(name="ps_mid", bufs=mid_bufs, space="PSUM") as ps_mid,
            tc.tile_pool(name="ps_po", bufs=po_bufs, space="PSUM") as ps_po,
        ):
            xh = cp.tile([P, 4, 2, S], fp8, tag="xh")
            xl = cp.tile([P, 4, 2, S], fp8, tag="xl")
            wqkh = cp.tile([P, 4, 2, 512], fp8, tag="wqkh")
            wqkl = cp.tile([P, 4, 2, 512], fp8, tag="wqkl")
            wvh = cp.tile([P, 4, 2, 256], fp8, tag="wvh")
            wvl = cp.tile([P, 4, 2, 256], fp8, tag="wvl")
            wo = cp.tile([P, 2, D], bf16, tag="wo")
            cosv = cp.tile([P, S], bf16, tag="cos")
            sinv = cp.tile([P, S], bf16, tag="sin")
            maskt = cp.tile([P, P], bf16, tag="mask")
            ident = cp.tile([P, P], fp16, tag="ident")
            qsb = cp.tile([P, 2, S], bf16, tag="qsb")
            ksb = cp.tile([P, 2, S], bf16, tag="ksb")
            v2 = cp.tile([P, NKT, HPC, 65], fp16, tag="v2")
            o2 = cp.tile([P, 2, S], fp16, tag="o2")

            # Act table warmup during the input-DMA wait
            warm = smallp.tile([P, 2], f32, tag="warm", name="warm")
            nc.vector.memset(warm[:], 0.0)
            nc.scalar.activation(warm[:, 0:1], warm[:, 1:2],
                                 Exp, scale=1.0)

            # input DMAs: few big transfers, first QKV chain's first
            sl0 = slice(0, QTILE)
            for kk in range(2):
                ksl = slice(kk * 2, kk * 2 + 2)
                nc.sync.dma_start(wqkh[:, ksl, :, :], wqkh_d[:, ksl, :, :])
                nc.sync.dma_start(xh[:, ksl, :, sl0], xh_d[:, ksl, :, sl0])
            for kk in range(2):
                ksl = slice(kk * 2, kk * 2 + 2)
                nc.sync.dma_start(xl[:, ksl, :, sl0], xl_d[:, ksl, :, sl0])
            nc.sync.dma_start(wqkl[:], wqkl_d[:])
            nc.sync.dma_start(cosv[:, sl0], cos_d[:, sl0])
            nc.sync.dma_start(sinv[:, sl0], sin_d[:, sl0])
            nc.sync.dma_start(wvh[:], wvh_d[:])
            nc.sync.dma_start(wvl[:], wvl_d[:])
            for t in range(1, NQ):
                sl = slice(t * QTILE, (t + 1) * QTILE)
                nc.sync.dma_start(xh[:, :, :, sl], xh_d[:, :, :, sl])
                nc.sync.dma_start(xl[:, :, :, sl], xl_d[:, :, :, sl])
                nc.sync.dma_start(cosv[:, sl], cos_d[:, sl])
                nc.sync.dma_start(sinv[:, sl], sin_d[:, sl])
                if t == 1:
                    nc.sync.dma_start(maskt[:], mask_d[:])
                    nc.sync.dma_start(ident[:], ident_d[:])
            nc.sync.dma_start(wo[:], wo_d[:])
            nc.gpsimd.memset(v2[:], WS)

            exp_n = [0]
            ysb_n = [0]

            def qkv_dr(out_ap, stat_tile_pair, mov_tile_pair, stat_sl, mov_sl):
                """12 DoubleRow matmuls: 3 residual products x 4 ksteps."""
                prods = [
                    (stat_tile_pair[0], mov_tile_pair[0]),
                    (stat_tile_pair[0], mov_tile_pair[1]),
                    (stat_tile_pair[1], mov_tile_pair[0]),
                ]
                n = len(prods) * 4
                i = 0
                for st, mv in prods:
                    for ks in range(4):
                        nc.tensor.matmul(
                            out_ap,
                            st[(slice(None), ks) + stat_sl],
                            mv[(slice(None), ks) + mov_sl],
                            start=(i == 0),
                            stop=(i == n - 1),
                            perf_mode=DR,
                        )
                        i += 1

            def qkv_unit(t, mt):
                """one QKV mt-tile (q or k head-pair) + its rope chain"""
                nsl = slice(t * QTILE, (t + 1) * QTILE)
                pqk = ps_mid.tile([P, QTILE], f32, tag="m", name="pqk")
                if "noqkv" in probe:
                    nc.vector.memset(pqk[:], 0.01)
                else:
                    qkv_dr(
                        pqk[:],
                        (wqkh, wqkl),
                        (xh, xl),
                        (slice(None), slice(mt * 128, (mt + 1) * 128)),
                        (slice(None), nsl),
                    )
                # rope: copy on Act, shuffle+muls+add on DVE (bf16 2x)
                cpy = ropep.tile([P, QTILE], bf16, tag="cpy", name="cpy")
                if xcopy_eng == "a":
                    nc.scalar.copy(cpy[:], pqk[:])
                else:
                    nc.vector.tensor_copy(cpy[:], pqk[:])
                sw = ropep.tile([P, QTILE], bf16, tag="sw", name="sw")
                nc.vector.stream_shuffle(sw[:], cpy[:], _PAIRSWAP)
                t0 = ropep.tile([P, QTILE], bf16, tag="t0", name="t0")
                nc.vector.tensor_mul(t0[:], cpy[:], cosv[:, nsl])
                u0 = ropep.tile([P, QTILE], bf16, tag="u0", name="u0")
                rm = nc.gpsimd if ropemul_eng == "p" else nc.vector
                rm.tensor_mul(u0[:], sw[:], sinv[:, nsl])
                dst = qsb if mt < 2 else ksb
                radd = nc.gpsimd if ropeadd_eng == "p" else nc.vector
                radd.tensor_add(dst[:, mt % 2, nsl], t0[:], u0[:])

            def v_unit(t, tt):
                kt = t * 4 + tt
                pv = ps_mid.tile([P, 2, HPC, DK], f32, tag="m", name="pv")
                if "noqkv" in probe:
                    nc.vector.memset(pv[:, 0, :, :], 0.01)
                else:
                    qkv_dr(
                        pv[:, 0, :, :],
                        (xh, xl),
                        (wvh, wvl),
                        (slice(None), slice(kt * P, (kt + 1) * P)),
                        (slice(None), slice(None)),
                    )
                if vcopy_eng == "a":
                    nc.scalar.copy(v2[:, kt, :, 0:DK], pv[:, 0, :, :])
                else:
                    nc.vector.tensor_copy(v2[:, kt, :, 0:DK], pv[:, 0, :, :])

            prabs = {}

            def sc_unit(qt, kt, hp):
                """scores+exp(+mask) for one (kt, head-pair)"""
                if True:
                    ktsl = slice(kt * P, (kt + 1) * P)
                    r = kt - 4 * qt  # 0..3 diag band; negative for full kts
                    rq = max(r, 0) * 128
                    qsl = slice(qt * QTILE + rq, (qt + 1) * QTILE)
                    if True:
                        pst = ps_sc.tile([P, 2, QTILE], f32, tag="s", name="pst")
                        if "noscores" in probe:
                            nc.vector.memset(pst[:, :, rq:rq + 1], 0.0)
                        else:
                            for half in range(2):
                                psl = slice(half * 64, half * 64 + 64)
                                nc.tensor.matmul(
                                    pst[:, half, rq:],
                                    ksb[psl, hp, ktsl],
                                    qsb[psl, hp, qsl],
                                    start=True,
                                    stop=True,
                                    tile_position=(half * 64, 0),
                                )
                        prab = probsp.tile([P, 2, QTILE], fp16, tag="pr", name="pr")
                        if diag_eng and r >= 0:
                            eng = diag_eng
                        else:
                            cyc = exp_sched.get(qt, exp_cycle)
                            eng = cyc[exp_n[0] % len(cyc)]
                            exp_n[0] += 1
                        if "noexp" in probe:
                            nc.vector.memset(prab[:, :, rq:rq + 1], 0.001)
                        elif eng == "a":
                            nc.scalar.activation(
                                prab[:, :, rq:], pst[:, :, rq:], Exp, scale=ESCALE
                            )
                        else:
                            e = nc.vector if eng == "d" else nc.gpsimd
                            e.tensor_scalar(
                                prab[:, :, rq:].bitcast(i16),
                                pst[:, :, rq:],
                                A16,
                                B16,
                                op0=mybir.AluOpType.mult,
                                op1=mybir.AluOpType.add,
                            )
                        if r >= 0 and "nomask" not in probe:
                            meng = nc.gpsimd if mask_eng == "p" else nc.vector
                            meng.tensor_mul(
                                prab[:, :, rq:rq + P],
                                prab[:, :, rq:rq + P],
                                maskt[:, None, :].to_broadcast([P, 2, P]),
                            )
                        prabs[(kt, hp)] = prab

            def pv_unit(qt, qi):
                # PV + normalize for one q128; sequential per-(q128, head)
                # chains so each po bank has one open PSUM group at a time
                if True:
                    c = qi % 4
                    po = ps_po.tile([P, HPC, 128], f32, tag="po", name="po")
                    if "nopv" in probe:
                        nc.vector.memset(po[:], 1.0)
                    else:
                        for h in range(HPC):
                            for kt in range(qi + 1):
                                nc.tensor.matmul(
                                    po[:, h, 0:65],
                                    prabs[(kt, h // 2)][
                                        :, h % 2, c * 128:(c + 1) * 128
                                    ],
                                    v2[:, kt, h, :],
                                    start=(kt == 0),
                                    stop=(kt == qi),
                                )
                    if "nonorm" in probe:
                        nc.vector.memset(o2[:, :, qi * P:(qi + 1) * P], 0.01)
                        return
                    rcp = smallp.tile([P, HPC], f32, tag="rc", name="rcp")
                    nc.vector.reciprocal(rcp[:], po[:, :, 64:65])
                    onorm = smallp.tile([P, HPC, DK], fp16, tag="on", name="onorm")
                    nc.vector.tensor_mul(
                        onorm[:],
                        po[:, :, 0:DK],
                        rcp[:, :, None].to_broadcast([P, HPC, DK]),
                    )
                    if "notr" in probe:
                        nc.vector.memset(o2[:, :, qi * P:(qi + 1) * P], 0.01)
                        return
                    tr = ps_mid.tile([P, 2, 256], f32, tag="m", name="tr")
                    trh = tr[:].bitcast(fp16)  # [P, 2, 512] fp16 view
                    for kj in range(2):
                        nc.tensor.transpose(
                            trh[:, kj, 0:128],
                            onorm[:, 2 * kj:2 * kj + 2, :],
                            ident[:],
                        )
                    if o2_eng == "a":
                        nc.scalar.copy(
                            o2[:, :, qi * P:(qi + 1) * P], trh[:, :, 0:128])
                    else:
                        nc.vector.tensor_copy(
                            o2[:, :, qi * P:(qi + 1) * P], trh[:, :, 0:128])

            def oproj_unit(t, ot, qsl=None):
                qsl = qsl or slice(t * QTILE, (t + 1) * QTILE)
                w = qsl.stop - qsl.start
                py = ps_mid.tile([P, QTILE], f32, tag="m", name="py")
                for kj in range(2):
                    nc.tensor.matmul(
                        py[:, 0:w],
                        wo[:, kj, ot * 128:(ot + 1) * 128],
                        o2[:, kj, qsl],
                        start=(kj == 0),
                        stop=(kj == 1),
                    )
                if "noy" in probe:
                    return
                ysb = ysbp.tile([P, QTILE], fp16, tag="y", name="ysb")
                eng = ysb_cycle[ysb_n[0] % len(ysb_cycle)]
                ysb_n[0] += 1
                if eng == "a":
                    nc.scalar.copy(ysb[:, 0:w], py[:, 0:w])
                elif eng == "d":
                    nc.vector.tensor_copy(ysb[:, 0:w], py[:, 0:w])
                else:
                    nc.gpsimd.tensor_copy(ysb[:, 0:w], py[:, 0:w])
                nc.sync.dma_start(
                    out_d[ot * 128:(ot + 1) * 128, qsl], ysb[:, 0:w])

            def body():
                if "noattn" in probe:
                    nc.vector.memset(o2[:], 0.01)
                # phase 0 QKV up front
                for mt in range(4):
                    qkv_unit(0, mt)
                for tt in range(4):
                    v_unit(0, tt)
                for t in range(NQ):
                    # fill units: next phase's QKV/V, prev phase's oproj
                    fill = []
                    qkvf = []
                    if t + 1 < NQ:
                        qkvf += [(qkv_unit, (t + 1, mt)) for mt in range(4)]
                        qkvf += [(v_unit, (t + 1, tt)) for tt in range(4)]
                    opf = []
                    if t > 0:
                        opf += [(oproj_unit, (t - 1, ot)) for ot in range(8)]
                    if fill_order == "qo":
                        fill = qkvf + opf
                    elif fill_order == "oq":
                        fill = opf + qkvf
                    else:  # interleaved
                        n = max(len(qkvf), len(opf))
                        for i in range(n):
                            if i < len(opf):
                                fill.append(opf[i])
                            if i < len(qkvf):
                                fill.append(qkvf[i])
                    # interleave: scores stream + PV chains + fill units
                    nkt = 4 * (t + 1)
                    fi = 0
                    emitted_pv = 0
                    if "noattn" in probe:
                        seq = []
                    else:
                        seq = [(sc_unit, (t, kt, hp))
                               for kt in range(nkt) for hp in range(2)]
                    for n, (fn, args) in enumerate(seq):
                        fn(*args)
                        kt = args[1]
                        # a fill unit after every sc pair
                        if n % 2 == 1 and fi < len(fill):
                            fn2, a2 = fill[fi]
                            fn2(*a2)
                            fi += 1
                        # PV chain for qi once sc(kt=qi+pvlag) has been emitted
                        while (emitted_pv < 4
                               and 4 * t + emitted_pv + pvlag <= kt):
                            pv_unit(t, 4 * t + emitted_pv)
                            emitted_pv += 1
                    if tail_order == "fp":
                        while fi < len(fill):
                            fn2, a2 = fill[fi]
                            fn2(*a2)
                            fi += 1
                    while emitted_pv < 4 and "noattn" not in probe:
                        pv_unit(t, 4 * t + emitted_pv)
                        emitted_pv += 1
                        if tail_order == "alt" and fi < len(fill):
                            fn2, a2 = fill[fi]
                            fn2(*a2)
                            fi += 1
                    while fi < len(fill):
                        fn2, a2 = fill[fi]
                        fn2(*a2)
                        fi += 1
                    prabs.clear()
                # last oproj
                for ot in range(8):
                    oproj_unit(NQ - 1, ot)

            if loop:
                with tc.For_i(0, reps, 1):
                    body()
            else:
                for _rep in range(reps):
                    body()
    nc.compile()
    return nc


def _f8(a):
    return np.asarray(a, dtype=F8NP)


def _dr_major(Wmat):
    """[M, 1024] -> [128, 4, 2, M]: (p, ks, i, m) = W[m, ks*256+i*128+p]."""
    M = Wmat.shape[0]
    return np.ascontiguousarray(
        Wmat.T.reshape(4, 2, P, M).transpose(2, 0, 1, 3)
    )


def _prep_in_maps(x, W_qkv, W_o, token_positions):
    x = np.asarray(x, dtype=np.float32)
    W_qkv = np.asarray(W_qkv, dtype=np.float32)
    W_o = np.asarray(W_o, dtype=np.float32)
    pos = np.asarray(token_positions)

    inv_freq = 1.0 / (
        np.float32(THETA) ** (np.arange(0, DK, 2, dtype=np.float32) / np.float32(DK))
    )
    freqs = pos.astype(np.float32)[:, :, None] * inv_freq[None, None, :]  # [B,S,32]
    cos = np.cos(freqs).astype(np.float32)
    sin = np.sin(freqs).astype(np.float32)
    jidx = (np.arange(P) % DK) // 2
    sign = np.where(np.arange(P) % 2 == 0, -1.0, 1.0).astype(np.float32)
    cos_tab = [np.ascontiguousarray(cos[b].T[jidx]).astype(BF) for b in range(B)]
    sin_tab = [
        np.ascontiguousarray(sin[b].T[jidx] * sign[:, None]).astype(BF)
        for b in range(B)
    ]

    masks = (np.arange(P)[:, None] <= np.arange(P)[None, :]).astype(BF)  # tril^T

    ident = np.eye(P, dtype=F16NP)

    # x residual split, DR layout
    xdr = []
    for b in range(B):
        xt = x[b].T.reshape(4, 2, P, S).transpose(2, 0, 1, 3)  # [128,4,2,S]
        xhi = _f8(xt)
        xlo = _f8(xt - xhi.astype(np.float32))
        xdr.append((np.ascontiguousarray(xhi), np.ascontiguousarray(xlo)))

    in_maps = []
    for c in range(N_CORES):
        b, hg = divmod(c, 4)
        heads = range(hg * HPC, (hg + 1) * HPC)
        q_rows = np.concatenate([W_qkv[h * DK:(h + 1) * DK] for h in heads])
        k_rows = np.concatenate(
            [W_qkv[D + h * DK:D + (h + 1) * DK] for h in heads]
        )
        v_rows = np.concatenate(
            [W_qkv[2 * D + h * DK:2 * D + (h + 1) * DK] for h in heads]
        )
        wqk = np.concatenate([q_rows, k_rows]) * WS  # [512, 1024]
        wv = v_rows * WS  # [256, 1024]
        wqk_t = _dr_major(wqk)
        wv_t = _dr_major(wv)
        wqkh = _f8(wqk_t)
        wqkl = _f8(wqk_t - wqkh.astype(np.float32))
        wvh = _f8(wv_t)
        wvl = _f8(wv_t - wvh.astype(np.float32))
        wo_sub = W_o[:, hg * 256:(hg + 1) * 256]  # [D, 256]
        wo = np.ascontiguousarray(
            wo_sub.T.reshape(2, P, D).transpose(1, 0, 2)
        ).astype(BF)
        in_maps.append(
            {
                "xh": xdr[b][0],
                "xl": xdr[b][1],
                "wqkh": np.ascontiguousarray(wqkh),
                "wqkl": np.ascontiguousarray(wqkl),
                "wvh": np.ascontiguousarray(wvh),
                "wvl": np.ascontiguousarray(wvl),
                "wo": wo,
                "cosv": cos_tab[b],
                "sinv": sin_tab[b],
                "masks": masks,
                "ident": ident,
            }
        )
    return in_maps


def _get_nc(reps=1, loop=False, probe=(), opts=None):
    key = f"nc{reps}_{loop}_{sorted(probe)}_{sorted((opts or {}).items())}"
    if key not in _CACHE:
        _CACHE[key] = _build_nc(reps, loop, probe, opts)
    return _CACHE[key]


def kernel(x, W_qkv, W_o, token_positions):
    nc = _get_nc()
    in_maps = _prep_in_maps(x, W_qkv, W_o, token_positions)
    res = run_bass_kernel_spmd(nc, in_maps, core_ids=list(range(N_CORES)))
    out = np.zeros((B, S, D), dtype=np.float32)
    for c in range(N_CORES):
        b = c // 4
        out[b] += np.asarray(res.results[c]["out_t"], dtype=np.float32).T
    return out
